# revision 8
# baseline (speedup 1.0000x reference)
"""CARAFE (content-aware upsample) Trainium2 kernel.

Sharding: 8 cores = batch(4) x H-halves(2). Host slices X with 2-row
zero-padded halos; each core computes its full output shard
[64, 128, 256]; host concatenates. No collectives.

Key algebraic simplification: dilation (2) == scale (2), so the
unfold patch for hi-res pixel (2h+r1, 2w+r2), tap (i,j) is
X[c, h+i-2, w+j-2] -- independent of the subpixel (r1,r2). The whole
CARAFE accumulation runs on the low-res grid:
  out_q[c,h,w] = sum_ij A[(i*5+j)*4+q, h, w] * X[c, h+i-2, w+j-2]
with A = softmax(pixel-shuffled encoder logits).

Implementation notes:
- conv path (1x1 + 3x3) runs in bf16 (PE streams 1 col/cycle) with
  fp32 PSUM accumulate; tolerance is 2e-2 so ~0.2% bf16 noise is fine.
- X^T is stored interleaved as Xt3[w, (row, j, c)] with the 5 j-shifted
  copies adjacent per row, which makes the 25 taps of the CARAFE sum a
  SINGLE stride-64 AP dim: per (row t, subpixel q) the apply stage is
  one rank-2 tensor_tensor product over all 25 taps and one
  reduce_sum(axis=X) -- two big DVE ops instead of 25 tiny ones.
- Everything after conv1 is emitted in 8-row blocks so the DVE apply
  work for block k overlaps PE/Scalar work for block k+1.
- Softmax: E = exp(logits) (O(1) logits, no max-subtraction), denom
  D via a strided DVE reduce of E^T, E^T scaled by 1/D per block.
"""

import numpy as np

SCALE = 2
KUP = 5
EPS = 1e-5
B, C, H, W = 4, 64, 128, 128
CMID = 64
ENC = 100  # (SCALE*KUP)**2
HALF = H // 2          # 64 low-res rows per core
HL = HALF + 4          # 68 rows of X incl. 2-row halos
WM1R = HALF + 2        # 66 rows of compressed features (1-row halo)
WM1W = W + 2           # 130 cols (1-col zero pad each side)
TB = 8                 # pipeline block: low-res rows per stage block
GP_Q = 0               # products per t offloaded to GpSimd (0..4)
REPS = 1               # in-NEFF repetitions (timing only; leave 1 for grading)


def _build_program():
    import concourse.bass as bass
    import concourse.tile as tile
    from concourse import mybir
    from concourse.vector_clock import ScopedClock

    f32 = mybir.dt.float32
    bf16 = mybir.dt.bfloat16

    class SplitDrainTC(tile.TileContext):
        # walrus in this container rejects >2 sync waits on one CTRL
        # instruction; put each tail-drain wait on its own SP nop.
        def _drain_and_barrier(self, tick_clock, wait_clock):
            probe = self.nc.sync.nop()
            wait_clock.add_sem_waits(
                probe.ins, ScopedClock({None: tick_clock.global_clock})
            )
            waits = list(probe.ins.sync_info.on_wait) if probe.ins.sync_info else []
            if probe.ins.sync_info:
                probe.ins.sync_info.on_wait = []
            for w in waits:
                n = self.nc.sync.nop()
                if n.ins.sync_info is None:
                    n.ins.sync_info = mybir.SyncInfo(on_wait=[w], on_update=[])
                else:
                    n.ins.sync_info.on_wait = [w]
            self.nc.sync.drain()
            self.nc.all_engine_barrier()
            assert self.sems is not None
            popped = self.nc._tile_sem_poison_stack.pop()
            assert popped is self._sem_poison
            self.nc.clear_and_free_semaphores(list(self.sems.allocated().values()))
            self.nc.all_engine_barrier()

    nc = bass.Bass()
    ap_in = {}
    for name, shape, dt in [
        ("Xh", [C, HL * W], bf16),
        ("W1", [C, CMID], bf16),
        ("W3", [C, 9 * ENC], bf16),
        ("c1s", [CMID, 1], f32),
        ("c1b", [CMID, 1], f32),
        ("c3s", [ENC, 1], f32),
        ("c3b", [ENC, 1], f32),
        ("ident", [128, 128], f32),
        ("identb", [128, 128], bf16),
    ]:
        ap_in[name] = nc.dram_tensor(name, shape, dt, kind="ExternalInput").ap()
    out_d = nc.dram_tensor("out", [C, SCALE * HALF, SCALE * W], f32,
                           kind="ExternalOutput").ap()

    mult = mybir.AluOpType.mult
    add = mybir.AluOpType.add
    AF = mybir.ActivationFunctionType

    with SplitDrainTC(nc) as tc:
        for _ in range(REPS):
            _build_tile_kernel(tc, nc, ap_in, out_d, mult, add, AF, bass, mybir)
    _split_sync_waits(nc, mybir)
    return nc


def _split_sync_waits(nc, mybir, max_waits=1):
    """walrus in this container rejects multiple sync waits on some
    instruction structs (Matmult allows just one);
    hoist the excess onto same-engine nops placed just before."""
    ctr = 0
    for bb in nc.m.functions[0].blocks:
        new = []
        changed = False
        for inst in bb.instructions:
            si = inst.sync_info
            waits = list(si.on_wait) if si and si.on_wait else []
            if len(waits) > max_waits:
                extra, keep = waits[:-max_waits], waits[-max_waits:]
                for i in range(0, len(extra), max_waits):
                    ctr += 1
                    nop = mybir.InstNoOp(name=f"wsplit-{ctr}", ins=[], outs=[])
                    nop.engine = inst.engine
                    nop.sync_info = mybir.SyncInfo(
                        on_wait=extra[i : i + max_waits], on_update=[]
                    )
                    new.append(nop)
                si.on_wait = keep
                changed = True
            new.append(inst)
        if changed:
            bb.instructions = new
    return ctr


def _build_tile_kernel(tc, nc, ap_in, out_d, mult, add, AF, bass, mybir):
    f32 = mybir.dt.float32
    bf16 = mybir.dt.bfloat16
    ctxs = []

    def pool(name, bufs, space="SBUF"):
        p = tc.tile_pool(name=name, bufs=bufs, space=space)
        ctxs.append(p)
        return p.__enter__()

    consts = pool("consts", 1)
    persist = pool("persist", 1)
    psA = pool("psA", 2, space="PSUM")     # conv matmul outputs
    psT = pool("psT", 2, space="PSUM")     # f32 transposes
    psB = pool("psB", 2, space="PSUM")     # bf16 transposes
    accp = pool("acc", 3)
    stagep = pool("stage", 3)
    prodp = pool("prod", 3)

    def A_(t, off, dims):
        return bass.AP(tensor=t.tensor, offset=t.offset + off, ap=[t.ap[0]] + dims)

    # ---- constants ----
    W1 = consts.tile([C, CMID], bf16, tag="w1")
    nc.sync.dma_start(W1[:], ap_in["W1"][:])
    W3 = consts.tile([C, 9 * ENC], bf16, tag="w3")
    nc.sync.dma_start(W3[:], ap_in["W3"][:])
    c1s = consts.tile([CMID, 1], f32, tag="c1s")
    nc.sync.dma_start(c1s[:], ap_in["c1s"][:])
    c1b = consts.tile([CMID, 1], f32, tag="c1b")
    nc.sync.dma_start(c1b[:], ap_in["c1b"][:])
    c3s = consts.tile([ENC, 1], f32, tag="c3s")
    nc.sync.dma_start(c3s[:], ap_in["c3s"][:])
    c3b = consts.tile([ENC, 1], f32, tag="c3b")
    nc.sync.dma_start(c3b[:], ap_in["c3b"][:])
    ident = consts.tile([128, 128], f32, tag="ident")
    nc.sync.dma_start(ident[:], ap_in["ident"][:])
    identb = consts.tile([128, 128], bf16, tag="identb")
    nc.sync.dma_start(identb[:], ap_in["identb"][:])

    # ---- load X (bf16) ----
    Xh = persist.tile([C, HL * W], bf16, tag="xh")
    nc.sync.dma_start(Xh[:], ap_in["Xh"][:])
    xh_v = Xh.rearrange("p (r w) -> p r w", w=W)

    # ---- compress: 1x1 conv + BN + ReLU -> Wm1 bf16 [C, 66 x 130] ----
    Wm1 = persist.tile([C, WM1R * WM1W], bf16, tag="wm1")
    wm_view = Wm1.rearrange("p (r w) -> p r w", w=WM1W)
    nc.gpsimd.memset(wm_view[:, :, 0:1], 0.0)
    nc.gpsimd.memset(wm_view[:, :, WM1W - 1 : WM1W], 0.0)
    r = 0
    while r < WM1R:
        rows = min(4, WM1R - r)
        n = rows * W
        ps = psA.tile([CMID, 512], f32, tag="ps")
        nc.tensor.matmul(
            ps[:, :n], W1[:], Xh[:, (r + 1) * W : (r + 1 + rows) * W],
            start=True, stop=True,
        )
        nc.scalar.activation(
            wm_view[:, r : r + rows, 1 : 1 + W],
            ps[:, :n].rearrange("p (r w) -> p r w", w=W),
            AF.Relu, bias=c1b[:], scale=c1s[:],
        )
        r += rows

    # ---- X transpose, interleaved: Xt3[w, (row 68, j 5, c 64)] bf16 ----
    # row stride 5*64=320, j stride 64, c stride 1: tap k=(i*5+j) of output
    # row t sits at offset (t+i)*320 + j*64 = t*320 + k*64 -- the 25 taps
    # are ONE stride-64 AP dim.
    Xt3 = persist.tile([128, HL * 5 * C], bf16, tag="xt3")
    xt3_v = Xt3.rearrange("p (r j c) -> p r j c", j=5, c=C)
    for rho in range(HL):
        ptb = psB.tile([128, 128], bf16, tag="pt")
        nc.tensor.transpose(ptb[:, :C], xh_v[:, rho, :], identb[:C, :C])
        nc.scalar.copy(xt3_v[:, rho, 2, :], ptb[:, :C])
    for j in [0, 1, 3, 4]:
        d = j - 2
        p0, p1 = max(0, -d), 128 - max(0, d)
        # zero the whole band first (gpsimd memset needs aligned start
        # partitions); the shift DMA then overwrites the interior.
        nc.gpsimd.memset(xt3_v[:, :, j, :], 0.0)
        nc.sync.dma_start(
            out=xt3_v[p0:p1, :, j, :], in_=xt3_v[p0 + d : p1 + d, :, 2, :]
        )

    # ---- persistent softmax tensors ----
    E = persist.tile([ENC, HALF * W], f32, tag="e")
    Et = persist.tile([128, HALF * ENC], f32, tag="et")
    et_v = Et.rearrange("p (t k) -> p t k", k=ENC)
    Dt = persist.tile([128, 4 * HALF], f32, tag="dt")
    Rt = persist.tile([128, 4 * HALF], f32, tag="rt")

    # ---- blocked pipeline: conv3+exp -> E^T -> 1/D -> norm -> apply ----
    for tb in range(0, HALF, TB):
        # 3x3 conv (bf16) + affine + exp -> E rows tb..tb+TB
        for t0 in range(tb, tb + TB, 4):
            ps = psA.tile([ENC, 512], f32, tag="ps")
            for ti, (di, dj) in enumerate(
                (di, dj) for di in range(3) for dj in range(3)
            ):
                off = (t0 + di) * WM1W + dj
                mv = A_(Wm1, off, [[WM1W, 4], [1, W]])
                nc.tensor.matmul(
                    ps[:], W3[:, ti * ENC : (ti + 1) * ENC], mv,
                    start=(ti == 0), stop=(ti == 8),
                )
            nc.scalar.activation(
                E[:, t0 * W : (t0 + 4) * W], ps[:], AF.Exp,
                bias=c3b[:], scale=c3s[:],
            )
        # E^T (pixel-major) for the block
        for t in range(tb, tb + TB):
            pt = psT.tile([128, 128], f32, tag="pt")
            nc.tensor.transpose(
                pt[:, :ENC], E[:, t * W : (t + 1) * W], ident[:ENC, :ENC]
            )
            nc.scalar.copy(et_v[:, t, :], pt[:, :ENC])
        # softmax denominators on DVE: D[w,(t,q)] = sum_k Et[w,t,4k+q]
        nc.vector.reduce_sum(
            A_(Dt, tb * 4, [[1, 4 * TB]]),
            A_(Et, tb * ENC, [[ENC, TB], [1, 4], [4, 25]]),
            axis=mybir.AxisListType.X,
        )
        nc.vector.reciprocal(A_(Rt, tb * 4, [[1, 4 * TB]]),
                             A_(Dt, tb * 4, [[1, 4 * TB]]))
        # normalize E^T in place for the block
        nc.vector.tensor_tensor(
            A_(Et, tb * ENC, [[1, TB * ENC]]),
            A_(Et, tb * ENC, [[1, TB * ENC]]),
            A_(Rt, tb * 4, [[4, TB], [0, 25], [1, 4]]),
            mult,
        )
        # ---- apply + pixel-shuffle writeout for the block ----
        # A/B experiment: per-q AP configs, compared via trace timings.
        #  q0,q1 (cfgA): product iter (taps,c) [fast write] + strided reduce
        #  q2    (cfgB): product iter (c,taps) [contig P] + fast reduce
        #  q3    (gpsimd, cfgB layout)
        for t in range(tb, tb + TB):
            stage = stagep.tile([C, 2 * 2 * W], f32, tag="stage")
            st_v = stage.rearrange("p (r x) -> p r x", r=2)
            acc4 = accp.tile([128, 4 * C], f32, tag="acc4")
            for q in range(4):
                P = prodp.tile([128, C * 25], f32, tag="prod")
                if q < 2:
                    x_ap = A_(Xt3, t * 5 * C, [[C, 25], [1, C]])
                    a_ap = A_(Et, t * ENC + q, [[4, 25], [0, C]])
                    p_out = A_(P, 0, [[C, 25], [1, C]])
                    red_in = A_(P, 0, [[1, C], [C, 25]])
                else:
                    x_ap = A_(Xt3, t * 5 * C, [[1, C], [C, 25]])
                    a_ap = A_(Et, t * ENC + q, [[0, C], [4, 25]])
                    p_out = A_(P, 0, [[25, C], [1, 25]])
                    red_in = A_(P, 0, [[25, C], [1, 25]])
                eng = nc.gpsimd if q == 3 else nc.vector
                eng.tensor_tensor(p_out, x_ap, a_ap, mult)
                nc.vector.reduce_sum(
                    A_(acc4, q * C, [[1, C]]), red_in,
                    axis=mybir.AxisListType.X,
                )
            for qp in range(2):
                po = psT.tile([128, 128], f32, tag="pt")
                nc.tensor.transpose(
                    po[:, :], acc4[:, qp * 128 : (qp + 1) * 128], ident[:, :]
                )
                for h in range(2):
                    q = qp * 2 + h
                    r1, r2 = q // 2, q % 2
                    out_ap = bass.AP(
                        tensor=st_v.tensor,
                        offset=st_v.offset + r1 * 2 * W + r2,
                        ap=[st_v.ap[0], [2, W]],
                    )
                    nc.scalar.copy(out_ap, po[h * C : (h + 1) * C, :])
            nc.sync.dma_start(out_d[:, 2 * t : 2 * t + 2, :], st_v)

    for p in reversed(ctxs):
        p.__exit__(None, None, None)


def _host_inputs(X, comp_w, comp_gamma, comp_beta, comp_mean, comp_var,
                 enc_w, enc_b, enc_gamma, enc_beta, enc_mean, enc_var):
    import ml_dtypes
    bf = ml_dtypes.bfloat16

    X = np.asarray(X, np.float32)
    inv1 = (np.asarray(comp_gamma, np.float32)
            / np.sqrt(np.asarray(comp_var, np.float32) + EPS))
    b1 = np.asarray(comp_beta, np.float32) - np.asarray(comp_mean, np.float32) * inv1
    inv3 = (np.asarray(enc_gamma, np.float32)
            / np.sqrt(np.asarray(enc_var, np.float32) + EPS))
    b3 = ((np.asarray(enc_b, np.float32) - np.asarray(enc_mean, np.float32)) * inv3
          + np.asarray(enc_beta, np.float32))

    W1 = np.ascontiguousarray(np.asarray(comp_w, np.float32)[:, :, 0, 0].T).astype(bf)
    # W3[c_in, tap*100 + c_out]
    W3 = np.ascontiguousarray(
        np.asarray(enc_w, np.float32).transpose(2, 3, 1, 0).reshape(9 * C, ENC)
        .reshape(9, C, ENC).transpose(1, 0, 2).reshape(C, 9 * ENC)
    ).astype(bf)
    ident = np.eye(128, dtype=np.float32)

    common = dict(
        W1=W1, W3=W3,
        c1s=inv1.reshape(CMID, 1), c1b=b1.reshape(CMID, 1),
        c3s=inv3.reshape(ENC, 1), c3b=b3.reshape(ENC, 1),
        ident=ident, identb=ident.astype(bf),
    )
    in_maps = []
    for s in range(8):
        b, half = divmod(s, 2)
        h0 = half * HALF
        xs = np.zeros((C, HL, W), np.float32)
        lo, hi = h0 - 2, h0 + HALF + 2
        clo, chi = max(lo, 0), min(hi, H)
        xs[:, clo - lo : clo - lo + (chi - clo), :] = X[b, :, clo:chi, :]
        in_maps.append(dict(Xh=xs.reshape(C, HL * W).astype(bf), **common))
    return in_maps


_PROGRAM_CACHE = {}


def _run(in_maps, trace=False, **kw):
    from concourse.bass_utils import run_bass_kernel_spmd

    if "nc" not in _PROGRAM_CACHE:
        _PROGRAM_CACHE["nc"] = _build_program()
    nc = _PROGRAM_CACHE["nc"]
    return run_bass_kernel_spmd(nc, in_maps, list(range(8)), trace=trace, **kw)


def _gather(res):
    out = np.zeros((B, C, SCALE * H, SCALE * W), np.float32)
    for s in range(8):
        b, half = divmod(s, 2)
        out[b, :, SCALE * half * HALF : SCALE * (half + 1) * HALF, :] = (
            res.results[s]["out"]
        )
    return out


def kernel(**inputs) -> np.ndarray:
    return _gather(_run(_host_inputs(**inputs)))


# revision 9
# speedup vs baseline: 1.2249x; 1.2249x over previous
"""CARAFE (content-aware upsample) Trainium2 kernel.

Sharding: 8 cores = batch(4) x H-halves(2). Host slices X with 2-row
zero-padded halos; each core computes its full output shard
[64, 128, 256]; host concatenates. No collectives.

Key algebraic simplification: dilation (2) == scale (2), so the
unfold patch for hi-res pixel (2h+r1, 2w+r2), tap (i,j) is
X[c, h+i-2, w+j-2] -- independent of the subpixel (r1,r2). The whole
CARAFE accumulation runs on the low-res grid:
  out_q[c,h,w] = sum_ij A[(i*5+j)*4+q, h, w] * X[c, h+i-2, w+j-2]
with A = softmax(pixel-shuffled encoder logits).

Implementation notes:
- conv path (1x1 + 3x3) runs in bf16 (PE streams 1 col/cycle) with
  fp32 PSUM accumulate; tolerance is 2e-2 so ~0.2% bf16 noise is fine.
- X^T is stored interleaved as Xt3[w, (row, j, c)] with the 5 j-shifted
  copies adjacent per row, which makes the 25 taps of the CARAFE sum a
  SINGLE stride-64 AP dim: per (row t, subpixel q) the apply stage is
  one rank-2 tensor_tensor product over all 25 taps and one
  reduce_sum(axis=X) -- two big DVE ops instead of 25 tiny ones.
- Everything after conv1 is emitted in 8-row blocks so the DVE apply
  work for block k overlaps PE/Scalar work for block k+1.
- Softmax: E = exp(logits) (O(1) logits, no max-subtraction), denom
  D via a strided DVE reduce of E^T, E^T scaled by 1/D per block.
"""

import numpy as np

SCALE = 2
KUP = 5
EPS = 1e-5
B, C, H, W = 4, 64, 128, 128
CMID = 64
ENC = 100  # (SCALE*KUP)**2
HALF = H // 2          # 64 low-res rows per core
HL = HALF + 4          # 68 rows of X incl. 2-row halos
WM1R = HALF + 2        # 66 rows of compressed features (1-row halo)
WM1W = W + 2           # 130 cols (1-col zero pad each side)
TB = 8                 # pipeline block: low-res rows per stage block
GP_Q = 0               # products per t offloaded to GpSimd (0..4)
REPS = 1               # in-NEFF repetitions (timing only; leave 1 for grading)


def _build_program():
    import concourse.bass as bass
    import concourse.tile as tile
    from concourse import mybir
    from concourse.vector_clock import ScopedClock

    f32 = mybir.dt.float32
    bf16 = mybir.dt.bfloat16

    class SplitDrainTC(tile.TileContext):
        # walrus in this container rejects >2 sync waits on one CTRL
        # instruction; put each tail-drain wait on its own SP nop.
        def _drain_and_barrier(self, tick_clock, wait_clock):
            probe = self.nc.sync.nop()
            wait_clock.add_sem_waits(
                probe.ins, ScopedClock({None: tick_clock.global_clock})
            )
            waits = list(probe.ins.sync_info.on_wait) if probe.ins.sync_info else []
            if probe.ins.sync_info:
                probe.ins.sync_info.on_wait = []
            for w in waits:
                n = self.nc.sync.nop()
                if n.ins.sync_info is None:
                    n.ins.sync_info = mybir.SyncInfo(on_wait=[w], on_update=[])
                else:
                    n.ins.sync_info.on_wait = [w]
            self.nc.sync.drain()
            self.nc.all_engine_barrier()
            assert self.sems is not None
            popped = self.nc._tile_sem_poison_stack.pop()
            assert popped is self._sem_poison
            self.nc.clear_and_free_semaphores(list(self.sems.allocated().values()))
            self.nc.all_engine_barrier()

    nc = bass.Bass()
    ap_in = {}
    for name, shape, dt in [
        ("Xh", [C, HL * W], bf16),
        ("W1", [C, CMID], bf16),
        ("W3", [C, 9 * ENC], bf16),
        ("c1s", [CMID, 1], f32),
        ("c1b", [CMID, 1], f32),
        ("c3s", [ENC, 1], f32),
        ("c3b", [ENC, 1], f32),
        ("ident", [128, 128], f32),
        ("identb", [128, 128], bf16),
    ]:
        ap_in[name] = nc.dram_tensor(name, shape, dt, kind="ExternalInput").ap()
    out_d = nc.dram_tensor("out", [C, SCALE * HALF, SCALE * W], f32,
                           kind="ExternalOutput").ap()

    mult = mybir.AluOpType.mult
    add = mybir.AluOpType.add
    AF = mybir.ActivationFunctionType

    with SplitDrainTC(nc) as tc:
        for _ in range(REPS):
            _build_tile_kernel(tc, nc, ap_in, out_d, mult, add, AF, bass, mybir)
    _split_sync_waits(nc, mybir)
    return nc


def _split_sync_waits(nc, mybir, max_waits=1):
    """walrus in this container rejects multiple sync waits on some
    instruction structs (Matmult allows just one);
    hoist the excess onto same-engine nops placed just before."""
    ctr = 0
    for bb in nc.m.functions[0].blocks:
        new = []
        changed = False
        for inst in bb.instructions:
            si = inst.sync_info
            waits = list(si.on_wait) if si and si.on_wait else []
            if len(waits) > max_waits:
                extra, keep = waits[:-max_waits], waits[-max_waits:]
                for i in range(0, len(extra), max_waits):
                    ctr += 1
                    nop = mybir.InstNoOp(name=f"wsplit-{ctr}", ins=[], outs=[])
                    nop.engine = inst.engine
                    nop.sync_info = mybir.SyncInfo(
                        on_wait=extra[i : i + max_waits], on_update=[]
                    )
                    new.append(nop)
                si.on_wait = keep
                changed = True
            new.append(inst)
        if changed:
            bb.instructions = new
    return ctr


def _build_tile_kernel(tc, nc, ap_in, out_d, mult, add, AF, bass, mybir):
    f32 = mybir.dt.float32
    bf16 = mybir.dt.bfloat16
    ctxs = []

    def pool(name, bufs, space="SBUF"):
        p = tc.tile_pool(name=name, bufs=bufs, space=space)
        ctxs.append(p)
        return p.__enter__()

    consts = pool("consts", 1)
    persist = pool("persist", 1)
    psA = pool("psA", 2, space="PSUM")     # conv matmul outputs
    psT = pool("psT", 2, space="PSUM")     # f32 transposes
    psB = pool("psB", 2, space="PSUM")     # bf16 transposes
    accp = pool("acc", 3)
    stagep = pool("stage", 3)
    prodp = pool("prod", 3)

    def A_(t, off, dims):
        return bass.AP(tensor=t.tensor, offset=t.offset + off, ap=[t.ap[0]] + dims)

    # ---- constants ----
    W1 = consts.tile([C, CMID], bf16, tag="w1")
    nc.sync.dma_start(W1[:], ap_in["W1"][:])
    W3 = consts.tile([C, 9 * ENC], bf16, tag="w3")
    nc.sync.dma_start(W3[:], ap_in["W3"][:])
    c1s = consts.tile([CMID, 1], f32, tag="c1s")
    nc.sync.dma_start(c1s[:], ap_in["c1s"][:])
    c1b = consts.tile([CMID, 1], f32, tag="c1b")
    nc.sync.dma_start(c1b[:], ap_in["c1b"][:])
    c3s = consts.tile([ENC, 1], f32, tag="c3s")
    nc.sync.dma_start(c3s[:], ap_in["c3s"][:])
    c3b = consts.tile([ENC, 1], f32, tag="c3b")
    nc.sync.dma_start(c3b[:], ap_in["c3b"][:])
    ident = consts.tile([128, 128], f32, tag="ident")
    nc.sync.dma_start(ident[:], ap_in["ident"][:])
    identb = consts.tile([128, 128], bf16, tag="identb")
    nc.sync.dma_start(identb[:], ap_in["identb"][:])

    # ---- load X (bf16) ----
    Xh = persist.tile([C, HL * W], bf16, tag="xh")
    nc.sync.dma_start(Xh[:], ap_in["Xh"][:])
    xh_v = Xh.rearrange("p (r w) -> p r w", w=W)

    # ---- compress: 1x1 conv + BN + ReLU -> Wm1 bf16 [C, 66 x 130] ----
    Wm1 = persist.tile([C, WM1R * WM1W], bf16, tag="wm1")
    wm_view = Wm1.rearrange("p (r w) -> p r w", w=WM1W)
    nc.gpsimd.memset(wm_view[:, :, 0:1], 0.0)
    nc.gpsimd.memset(wm_view[:, :, WM1W - 1 : WM1W], 0.0)
    r = 0
    while r < WM1R:
        rows = min(4, WM1R - r)
        n = rows * W
        ps = psA.tile([CMID, 512], f32, tag="ps")
        nc.tensor.matmul(
            ps[:, :n], W1[:], Xh[:, (r + 1) * W : (r + 1 + rows) * W],
            start=True, stop=True,
        )
        nc.scalar.activation(
            wm_view[:, r : r + rows, 1 : 1 + W],
            ps[:, :n].rearrange("p (r w) -> p r w", w=W),
            AF.Relu, bias=c1b[:], scale=c1s[:],
        )
        r += rows

    # ---- X transpose, interleaved: Xt3[w, (row 68, j 5, c 64)] bf16 ----
    # row stride 5*64=320, j stride 64, c stride 1: tap k=(i*5+j) of output
    # row t sits at offset (t+i)*320 + j*64 = t*320 + k*64 -- the 25 taps
    # are ONE stride-64 AP dim.
    Xt3 = persist.tile([128, HL * 5 * C], bf16, tag="xt3")
    xt3_v = Xt3.rearrange("p (r j c) -> p r j c", j=5, c=C)
    for rho in range(HL):
        ptb = psB.tile([128, 128], bf16, tag="pt")
        nc.tensor.transpose(ptb[:, :C], xh_v[:, rho, :], identb[:C, :C])
        nc.scalar.copy(xt3_v[:, rho, 2, :], ptb[:, :C])
    for j in [0, 1, 3, 4]:
        d = j - 2
        p0, p1 = max(0, -d), 128 - max(0, d)
        # zero the whole band first (gpsimd memset needs aligned start
        # partitions); the shift DMA then overwrites the interior.
        nc.gpsimd.memset(xt3_v[:, :, j, :], 0.0)
        nc.sync.dma_start(
            out=xt3_v[p0:p1, :, j, :], in_=xt3_v[p0 + d : p1 + d, :, 2, :]
        )

    # ---- persistent softmax tensors ----
    E = persist.tile([ENC, HALF * W], f32, tag="e")
    Et = persist.tile([128, HALF * ENC], f32, tag="et")
    et_v = Et.rearrange("p (t k) -> p t k", k=ENC)
    Dt = persist.tile([128, 4 * HALF], f32, tag="dt")
    Rt = persist.tile([128, 4 * HALF], f32, tag="rt")

    # ---- blocked pipeline: conv3+exp -> E^T -> 1/D -> norm -> apply ----
    for tb in range(0, HALF, TB):
        # 3x3 conv (bf16) + affine + exp -> E rows tb..tb+TB
        for t0 in range(tb, tb + TB, 4):
            ps = psA.tile([ENC, 512], f32, tag="ps")
            for ti, (di, dj) in enumerate(
                (di, dj) for di in range(3) for dj in range(3)
            ):
                off = (t0 + di) * WM1W + dj
                mv = A_(Wm1, off, [[WM1W, 4], [1, W]])
                nc.tensor.matmul(
                    ps[:], W3[:, ti * ENC : (ti + 1) * ENC], mv,
                    start=(ti == 0), stop=(ti == 8),
                )
            nc.scalar.activation(
                E[:, t0 * W : (t0 + 4) * W], ps[:], AF.Exp,
                bias=c3b[:], scale=c3s[:],
            )
        # E^T (pixel-major) for the block
        for t in range(tb, tb + TB):
            pt = psT.tile([128, 128], f32, tag="pt")
            nc.tensor.transpose(
                pt[:, :ENC], E[:, t * W : (t + 1) * W], ident[:ENC, :ENC]
            )
            nc.scalar.copy(et_v[:, t, :], pt[:, :ENC])
        # softmax denominators on DVE: D[w,(t,q)] = sum_k Et[w,t,4k+q]
        nc.vector.reduce_sum(
            A_(Dt, tb * 4, [[1, 4 * TB]]),
            A_(Et, tb * ENC, [[ENC, TB], [1, 4], [4, 25]]),
            axis=mybir.AxisListType.X,
        )
        nc.vector.reciprocal(A_(Rt, tb * 4, [[1, 4 * TB]]),
                             A_(Dt, tb * 4, [[1, 4 * TB]]))
        # normalize E^T in place for the block
        nc.vector.tensor_tensor(
            A_(Et, tb * ENC, [[1, TB * ENC]]),
            A_(Et, tb * ENC, [[1, TB * ENC]]),
            A_(Rt, tb * 4, [[4, TB], [0, 25], [1, 4]]),
            mult,
        )
        # ---- apply + pixel-shuffle writeout for the block ----
        # Per (t,q): one flat-coalescing product (iteration (taps,c): all
        # APs merge to rank<=2, in1 broadcast over c) + one strided reduce.
        # NOTE: GpSimd offload is a pessimization -- it shares the DVE's
        # SBUF ports, so concurrent gpsimd ops slow DVE ops ~1.6x.
        for t in range(tb, tb + TB):
            stage = stagep.tile([C, 2 * 2 * W], f32, tag="stage")
            st_v = stage.rearrange("p (r x) -> p r x", r=2)
            acc4 = accp.tile([128, 4 * C], f32, tag="acc4")
            for q in range(4):
                P = prodp.tile([128, C * 25], f32, tag="prod")
                nc.vector.tensor_tensor(
                    A_(P, 0, [[C, 25], [1, C]]),
                    A_(Xt3, t * 5 * C, [[C, 25], [1, C]]),
                    A_(Et, t * ENC + q, [[4, 25], [0, C]]),
                    mult,
                )
                nc.vector.reduce_sum(
                    A_(acc4, q * C, [[1, C]]),
                    A_(P, 0, [[1, C], [C, 25]]),
                    axis=mybir.AxisListType.X,
                )
            for qp in range(2):
                po = psT.tile([128, 128], f32, tag="pt")
                nc.tensor.transpose(
                    po[:, :], acc4[:, qp * 128 : (qp + 1) * 128], ident[:, :]
                )
                for h in range(2):
                    q = qp * 2 + h
                    r1, r2 = q // 2, q % 2
                    out_ap = bass.AP(
                        tensor=st_v.tensor,
                        offset=st_v.offset + r1 * 2 * W + r2,
                        ap=[st_v.ap[0], [2, W]],
                    )
                    nc.scalar.copy(out_ap, po[h * C : (h + 1) * C, :])
            nc.sync.dma_start(out_d[:, 2 * t : 2 * t + 2, :], st_v)

    for p in reversed(ctxs):
        p.__exit__(None, None, None)


def _host_inputs(X, comp_w, comp_gamma, comp_beta, comp_mean, comp_var,
                 enc_w, enc_b, enc_gamma, enc_beta, enc_mean, enc_var):
    import ml_dtypes
    bf = ml_dtypes.bfloat16

    X = np.asarray(X, np.float32)
    inv1 = (np.asarray(comp_gamma, np.float32)
            / np.sqrt(np.asarray(comp_var, np.float32) + EPS))
    b1 = np.asarray(comp_beta, np.float32) - np.asarray(comp_mean, np.float32) * inv1
    inv3 = (np.asarray(enc_gamma, np.float32)
            / np.sqrt(np.asarray(enc_var, np.float32) + EPS))
    b3 = ((np.asarray(enc_b, np.float32) - np.asarray(enc_mean, np.float32)) * inv3
          + np.asarray(enc_beta, np.float32))

    W1 = np.ascontiguousarray(np.asarray(comp_w, np.float32)[:, :, 0, 0].T).astype(bf)
    # W3[c_in, tap*100 + c_out]
    W3 = np.ascontiguousarray(
        np.asarray(enc_w, np.float32).transpose(2, 3, 1, 0).reshape(9 * C, ENC)
        .reshape(9, C, ENC).transpose(1, 0, 2).reshape(C, 9 * ENC)
    ).astype(bf)
    ident = np.eye(128, dtype=np.float32)

    common = dict(
        W1=W1, W3=W3,
        c1s=inv1.reshape(CMID, 1), c1b=b1.reshape(CMID, 1),
        c3s=inv3.reshape(ENC, 1), c3b=b3.reshape(ENC, 1),
        ident=ident, identb=ident.astype(bf),
    )
    in_maps = []
    for s in range(8):
        b, half = divmod(s, 2)
        h0 = half * HALF
        xs = np.zeros((C, HL, W), np.float32)
        lo, hi = h0 - 2, h0 + HALF + 2
        clo, chi = max(lo, 0), min(hi, H)
        xs[:, clo - lo : clo - lo + (chi - clo), :] = X[b, :, clo:chi, :]
        in_maps.append(dict(Xh=xs.reshape(C, HL * W).astype(bf), **common))
    return in_maps


_PROGRAM_CACHE = {}


def _run(in_maps, trace=False, **kw):
    from concourse.bass_utils import run_bass_kernel_spmd

    if "nc" not in _PROGRAM_CACHE:
        _PROGRAM_CACHE["nc"] = _build_program()
    nc = _PROGRAM_CACHE["nc"]
    return run_bass_kernel_spmd(nc, in_maps, list(range(8)), trace=trace, **kw)


def _gather(res):
    out = np.zeros((B, C, SCALE * H, SCALE * W), np.float32)
    for s in range(8):
        b, half = divmod(s, 2)
        out[b, :, SCALE * half * HALF : SCALE * (half + 1) * HALF, :] = (
            res.results[s]["out"]
        )
    return out


def kernel(**inputs) -> np.ndarray:
    return _gather(_run(_host_inputs(**inputs)))
